# revision 12
# baseline (speedup 1.0000x reference)
"""Gated linear attention (GLA) forward on 8 TRN2 NeuronCores via Bass/Tile.

Sharding: tensor-parallel over heads. Core c owns heads [2c, 2c+1]
(hidden columns [128c, 128c+128)). Projections are column-parallel;
the out-projection is column-parallel in the *output* dim, fed by an
in-kernel AllGather of the normalized/gated scan output.

Per-core pipeline (all on device):
  x[4096,1024] --(cast bf16 + DMA-transpose)--> x^T chunks
  q,k,v,gate,og,d projections in transposed (hidden-major) form
  chunked GLA scan (chunk=128) with the decay matrix materialized from
  the per-head scalar log-decay (D_ij = exp(G_i - G_j), masked)
  rms-norm partial sums + out-gate, AllGather(z^T || sumsq) in bf16
  y[:, 128c:128c+128] = (z * rsqrt(mean sq)) @ Wo_c^T
"""
import numpy as np

B, S, DM, H, DH = 2, 2048, 1024, 16, 64
HID = H * DH
NCORE = 8
HL = H // NCORE          # heads per core = 2
CW = HL * DH             # hidden cols per core = 128
M = B * S                # 4096 rows
C = 128                  # scan chunk length
T = S // C               # chunks per batch = 16
NU = B * HL * T          # chunk-units per core = 64
KB = DM // 128           # contraction chunks = 8
MBS = 512                # m-block for projections
NMB = M // MBS           # 8
NMT = M // 128           # 32 m-tiles
EPS = float(np.finfo(np.float32).eps)
MASK_NEG = -30000.0

_CACHE = {}


def _build_nc():
    import concourse.bass as bass
    import concourse.mybir as mybir
    from concourse import bacc
    from concourse.tile import TileContext
    from concourse.masks import make_identity, make_upper_triangular, make_lower_triangular

    f32 = mybir.dt.float32
    bf16 = mybir.dt.bfloat16
    FT = mybir.ActivationFunctionType
    OP = mybir.AluOpType

    nc = bacc.Bacc("TRN2", target_bir_lowering=False, num_devices=NCORE)

    x_d = nc.dram_tensor("x", [M, DM], f32, kind="ExternalInput")
    wq_d = nc.dram_tensor("wq", [CW, DM], f32, kind="ExternalInput")
    wk_d = nc.dram_tensor("wk", [CW, DM], f32, kind="ExternalInput")
    wv_d = nc.dram_tensor("wv", [CW, DM], f32, kind="ExternalInput")
    wg_d = nc.dram_tensor("wg", [CW, DM], f32, kind="ExternalInput")
    wog_d = nc.dram_tensor("wog", [CW, DM], f32, kind="ExternalInput")
    wo_d = nc.dram_tensor("wo", [CW, DM], f32, kind="ExternalInput")
    wdt_d = nc.dram_tensor("wdt", [DM, HL], f32, kind="ExternalInput")
    bd_d = nc.dram_tensor("bd", [HL, 1], f32, kind="ExternalInput")
    nw_d = nc.dram_tensor("nw", [CW, 1], f32, kind="ExternalInput")
    y_d = nc.dram_tensor("y", [M, CW], f32, kind="ExternalOutput")

    gsp_d = nc.dram_tensor("gsp", [HL, M], f32, kind="Internal")
    grow_d = nc.dram_tensor("grow", [NU, 128], f32, kind="Internal")
    AGC = M + NMT  # 4128: z^T columns ++ per-chunk sumsq columns
    agin_d = nc.dram_tensor("agin", [CW, AGC], bf16, kind="Internal")
    agout_d = nc.dram_tensor("agout", [NCORE * CW, AGC], bf16, kind="Internal",
                             addr_space="Shared")

    with TileContext(nc) as tc:
        with tc.tile_pool(name="const", bufs=1) as cpool, \
             tc.tile_pool(name="persist", bufs=1) as pp, \
             tc.tile_pool(name="wload", bufs=2) as wl, \
             tc.tile_pool(name="xload", bufs=2) as xl, \
             tc.tile_pool(name="xtp", bufs=2) as xtp, \
             tc.tile_pool(name="evac", bufs=2) as ev, \
             tc.tile_pool(name="scan", bufs=3) as sc, \
             tc.tile_pool(name="state", bufs=2) as stp, \
             tc.tile_pool(name="zload", bufs=8) as zl, \
             tc.tile_pool(name="fin", bufs=2) as fin, \
             tc.tile_pool(name="spp", bufs=1) as spp, \
             tc.tile_pool(name="ps_proj", bufs=2, space="PSUM") as ps_proj, \
             tc.tile_pool(name="ps_d", bufs=1, space="PSUM") as ps_d, \
             tc.tile_pool(name="ps_a", bufs=2, space="PSUM") as ps_a, \
             tc.tile_pool(name="ps_o", bufs=2, space="PSUM") as ps_o, \
             tc.tile_pool(name="ps_kv", bufs=1, space="PSUM") as ps_kv:

            # ---- constants -------------------------------------------------
            ident = cpool.tile([128, 128], f32)
            make_identity(nc, ident)
            lincl = cpool.tile([128, 128], f32)   # -1 where j <= i
            make_upper_triangular(nc, lincl, val=-1.0, diag=True)
            lrev = cpool.tile([128, 128], f32)    # -1 where j > i
            make_lower_triangular(nc, lrev, val=-1.0, diag=False)
            maskc = cpool.tile([128, 128], f32)   # 0 where j <= i else -3e4
            nc.gpsimd.memset(maskc, 0.0)
            # keep where (i - j) >= 0, fill future (j > i) with MASK_NEG
            nc.gpsimd.affine_select(
                out=maskc, in_=maskc, compare_op=mybir.AluOpType.is_ge,
                fill=MASK_NEG, base=0, pattern=[[1, 128]], channel_multiplier=-1)
            onesc = cpool.tile([128, 1], f32)
            nc.gpsimd.memset(onesc, 1.0)
            epsc = cpool.tile([128, 1], f32)
            nc.gpsimd.memset(epsc, EPS)

            bd_sb = cpool.tile([HL, 1], f32)
            nc.sync.dma_start(out=bd_sb, in_=bd_d[:, :])
            nw_sb = cpool.tile([CW, 1], f32)
            nc.sync.dma_start(out=nw_sb, in_=nw_d[:, :])

            # ---- weights: cast to bf16 and transpose to [k-part, kb, n] ----
            def load_wT(dram):
                wsb = wl.tile([CW, DM], bf16, tag="wsb")
                nc.gpsimd.dma_start(out=wsb, in_=dram[:, :])
                wT = pp.tile([128, KB, CW], bf16, tag=f"wT{dram.name}")
                nc.sync.dma_start(out=wT, in_=wsb, transpose=True)
                return wT

            wqT = load_wT(wq_d)
            wkT = load_wT(wk_d)
            wvT = load_wT(wv_d)
            wgT = load_wT(wg_d)
            wogT = load_wT(wog_d)
            woT = load_wT(wo_d)
            wdT = pp.tile([128, KB, HL], bf16, tag="wdT")
            for kb in range(KB):
                nc.gpsimd.dma_start(out=wdT[:, kb, :],
                                    in_=wdt_d[kb * 128:(kb + 1) * 128, :])

            # ---- persistent activation buffers -----------------------------
            QT = pp.tile([CW, M], bf16, tag="QT")
            KgT = pp.tile([CW, M], bf16, tag="KgT")
            VT = pp.tile([CW, M], bf16, tag="VT")
            OgsT = pp.tile([CW, M], bf16, tag="OgsT")
            oT = pp.tile([CW, M], bf16, tag="oT")

            # ---- x load + transpose + projections, per m-block -------------
            for mb in range(NMB):
                xT = xtp.tile([128, KB, MBS], bf16, tag="xT")
                for q in range(MBS // 128):
                    mt = mb * (MBS // 128) + q
                    xb = xl.tile([128, DM], bf16, tag="xb")
                    nc.gpsimd.dma_start(
                        out=xb, in_=x_d[mt * 128:(mt + 1) * 128, :])
                    nc.sync.dma_start(
                        out=xT[:, :, q * 128:(q + 1) * 128], in_=xb,
                        transpose=True)

                ms = slice(mb * MBS, (mb + 1) * MBS)

                def proj(wT, tag):
                    ps = ps_proj.tile([128, MBS], f32, tag="proj")
                    for kb in range(KB):
                        nc.tensor.matmul(ps, lhsT=wT[:, kb, :],
                                         rhs=xT[:, kb, :],
                                         start=(kb == 0), stop=(kb == KB - 1))
                    return ps

                pq = proj(wqT, "q")
                nc.vector.tensor_copy(QT[:, ms], pq)
                pk = proj(wkT, "k")
                kbf = ev.tile([128, MBS], bf16, tag="kbf")
                nc.vector.tensor_copy(kbf, pk)
                pg = proj(wgT, "g")
                # sigmoid(z) = 1 / (1 + exp(-z)) -- keeps ACT on the exp/ln table
                gate = ev.tile([128, MBS], f32, tag="gate")
                nc.scalar.activation(gate, pg, FT.Exp, scale=-1.0)
                nc.vector.tensor_scalar(gate, gate, 1.0, None, OP.add)
                nc.vector.reciprocal(gate, gate)
                nc.vector.tensor_tensor(KgT[:, ms], kbf, gate, OP.mult)
                pv = proj(wvT, "v")
                nc.vector.tensor_copy(VT[:, ms], pv)
                pog = proj(wogT, "og")
                ogt = ev.tile([128, MBS], f32, tag="ogt")
                nc.scalar.activation(ogt, pog, FT.Exp, scale=-1.0)
                nc.vector.tensor_scalar(ogt, ogt, 1.0, None, OP.add)
                nc.vector.reciprocal(ogt, ogt)
                nc.vector.tensor_copy(OgsT[:, ms], ogt)

                # d-projection (tiny output, shares moving operand)
                pd = ps_d.tile([HL, MBS], f32, tag="pd")
                for kb in range(KB):
                    nc.tensor.matmul(pd, lhsT=wdT[:, kb, :], rhs=xT[:, kb, :],
                                     start=(kb == 0), stop=(kb == KB - 1))
                # softplus(d + bd) = max(z,0) + ln(1 + exp(-|z|))
                zd = ev.tile([HL, MBS], f32, tag="zd")
                nc.vector.tensor_scalar(zd, pd, bd_sb[:, 0:1], None, OP.add)
                za = ev.tile([HL, MBS], f32, tag="za")
                nc.vector.tensor_scalar(za, zd, 0.0, None, OP.max)
                nc.scalar.activation(zd, zd, FT.Abs)
                nc.scalar.activation(zd, zd, FT.Exp, scale=-1.0)
                nc.scalar.activation(zd, zd, FT.Ln, bias=onesc[0:HL, 0:1])
                nc.vector.tensor_tensor(zd, zd, za, OP.add)
                nc.sync.dma_start(out=gsp_d[:, ms], in_=zd)

            # ---- G machinery ----------------------------------------------
            gcol = pp.tile([128, NU], f32, tag="gcol")  # col u=(b*2+h)*16+t
            for b in range(B):
                for h in range(HL):
                    u0 = (b * HL + h) * T
                    nc.sync.dma_start(
                        out=gcol[:, u0:u0 + T],
                        in_=gsp_d[h:h + 1, b * S:(b + 1) * S].rearrange(
                            "one (t p) -> p (one t)", p=128))

            gps = ps_a.tile([128, NU], f32, tag="aps")
            nc.tensor.matmul(gps, lhsT=lincl, rhs=gcol, start=True, stop=True)
            Gcol = pp.tile([128, NU], f32, tag="Gcol")      # G (negative)
            nc.vector.tensor_copy(Gcol, gps)
            nGcol = pp.tile([128, NU], f32, tag="nGcol")    # -G
            nc.vector.tensor_scalar(nGcol, Gcol, -1.0, None, OP.mult)
            grps = ps_a.tile([128, NU], f32, tag="aps")
            nc.tensor.matmul(grps, lhsT=lrev, rhs=gcol, start=True, stop=True)
            erev = pp.tile([128, NU], f32, tag="erev")      # exp(G_C - G_j)
            nc.scalar.activation(erev, grps, FT.Exp)
            trps = ps_o.tile([NU, 128], f32, tag="ops")
            nc.tensor.transpose(trps, Gcol, ident)
            Grow = pp.tile([NU, 128], f32, tag="Grow")
            nc.vector.tensor_copy(Grow, trps)
            nc.sync.dma_start(out=grow_d[:, :], in_=Grow)
            # e^{G_C} replicated over 64 partitions (for state decay).
            # G_C = G[0] + Grev[0] (inclusive-from-0 + strictly-after-0).
            gc0 = pp.tile([1, NU], f32, tag="gc0")
            nc.vector.tensor_tensor(gc0, Gcol[0:1, :], grps[0:1, :], OP.add)
            GCrep = pp.tile([64, NU], f32, tag="GCrep")
            nc.gpsimd.partition_broadcast(GCrep, gc0[0:1, :])
            eGC = pp.tile([64, NU], f32, tag="eGC")
            nc.scalar.activation(eGC, GCrep, FT.Exp)

            # ---- chunked scan ----------------------------------------------
            states = [None] * (B * HL)
            for t in range(T):
                for b in range(B):
                    cols = slice(b * S + t * C, b * S + (t + 1) * C)
                    Vn = sc.tile([128, 128], bf16, tag="Vn")
                    nc.sync.dma_start(out=Vn, in_=VT[:, cols], transpose=True)
                    Kgn = sc.tile([128, 128], bf16, tag="Kgn")
                    nc.sync.dma_start(out=Kgn, in_=KgT[:, cols], transpose=True)
                    ops = ps_o.tile([128, 128], f32, tag="ops")
                    us = [(b * HL + h) * T + t for h in range(HL)]
                    # decay-row broadcasts, one per head
                    greps = []
                    for h in range(HL):
                        grep = sc.tile([128, 128], f32, tag=f"grep{h}")
                        nc.sync.dma_start(
                            out=grep,
                            in_=grow_d[us[h]:us[h] + 1, :].to_broadcast(
                                (128, 128)))
                        greps.append(grep)
                    # q~^T for both heads at base partition 0: rows
                    # [64h, 64h+64) scaled by exp(G of head h's unit)
                    eg2 = sc.tile([128, 128], f32, tag="eg2")
                    for h in range(HL):
                        hs = slice(h * DH, (h + 1) * DH)
                        nc.scalar.activation(eg2[hs, :], greps[h][hs, :],
                                             FT.Exp)
                    qs2 = sc.tile([128, 128], bf16, tag="qs2")
                    nc.vector.tensor_tensor(qs2, QT[:, cols], eg2, OP.mult)
                    sbf2 = sc.tile([128, 64], bf16, tag="sbf2")
                    if t > 0:
                        for h in range(HL):
                            hs = slice(h * DH, (h + 1) * DH)
                            nc.vector.tensor_copy(sbf2[hs, :],
                                                  states[b * HL + h])
                    for h in range(HL):
                        u = us[h]
                        lane = b * HL + h
                        hs = slice(h * DH, (h + 1) * DH)
                        # A^T[j, i] (raw scores)
                        aps = ps_a.tile([128, 128], f32, tag="aps")
                        nc.tensor.matmul(aps, lhsT=KgT[hs, cols],
                                         rhs=QT[hs, cols], start=True, stop=True)
                        # decay matrix D^T[j, i] = exp(G_i - G_j) masked
                        dexp = sc.tile([128, 128], f32, tag="dexp")
                        nc.vector.tensor_tensor(dexp, greps[h], maskc, OP.add)
                        dmat = sc.tile([128, 128], f32, tag="dmat")
                        nc.scalar.activation(dmat, dexp, FT.Exp,
                                             bias=nGcol[:, u:u + 1])
                        abf = sc.tile([128, 128], bf16, tag="abf")
                        nc.vector.tensor_tensor(abf, aps, dmat, OP.mult)
                        # o^T[dv, i] = V^T A + S_old^T q~^T  (packed 2 heads)
                        nc.tensor.matmul(ops[hs, :], lhsT=Vn[:, hs], rhs=abf,
                                         start=True, stop=(t == 0))
                        if t > 0:
                            nc.tensor.matmul(ops[hs, :], lhsT=sbf2[hs, :],
                                             rhs=qs2[hs, :],
                                             start=False, stop=True)
                        # state update: S_new = e^{G_C} S_old + K~^T V
                        ksc = sc.tile([128, 64], bf16, tag="ksc")
                        nc.vector.tensor_scalar(ksc, Kgn[:, hs],
                                                erev[:, u:u + 1], None, OP.mult)
                        kvps = ps_kv.tile([64, 64], f32, tag="kvps")
                        nc.tensor.matmul(kvps, lhsT=ksc, rhs=Vn[:, hs],
                                         start=True, stop=True)
                        snew = stp.tile([64, 64], f32, tag=f"st{lane}")
                        if t == 0:
                            nc.vector.tensor_copy(snew, kvps)
                        else:
                            stmp = sc.tile([64, 64], f32, tag="stmp")
                            nc.vector.tensor_scalar(stmp, states[lane],
                                                    eGC[:, u:u + 1], None,
                                                    OP.mult)
                            nc.vector.tensor_tensor(snew, stmp, kvps, OP.add)
                        states[lane] = snew
                    nc.vector.tensor_copy(oT[:, cols], ops)

            # ---- sumsq partials + z^T, feed AllGather ----------------------
            ssqT = pp.tile([128, NMT], bf16, tag="ssqT")
            for mt in range(NMT):
                mts = slice(mt * 128, (mt + 1) * 128)
                zt = fin.tile([128, 128], bf16, tag="zt")
                nc.vector.tensor_tensor(zt, oT[:, mts], OgsT[:, mts], OP.mult)
                nc.vector.tensor_scalar(zt, zt, nw_sb[:, 0:1], None, OP.mult)
                nc.sync.dma_start(out=agin_d[:, mts], in_=zt)
                osq = fin.tile([128, 128], f32, tag="osq")
                nc.scalar.activation(osq, oT[:, mts], FT.Square)
                sps = ps_kv.tile([128, 1], f32, tag="kvps")
                nc.tensor.matmul(sps, lhsT=osq, rhs=onesc, start=True,
                                 stop=True)
                nc.vector.tensor_copy(ssqT[:, mt:mt + 1], sps)
            nc.sync.dma_start(out=agin_d[:, M:AGC], in_=ssqT)

            nc.gpsimd.collective_compute(
                "AllGather", mybir.AluOpType.bypass,
                replica_groups=[list(range(NCORE))],
                ins=[agin_d[:, :]], outs=[agout_d[:, :]])

            # ---- rsqrt(mean sq) --------------------------------------------
            sparts = spp.tile([128, NCORE, NMT], f32, tag="sparts")
            for r in range(NCORE):
                nc.gpsimd.dma_start(
                    out=sparts[:, r, :],
                    in_=agout_d[r * CW:r * CW + 128, M:AGC])
            for step in (4, 2, 1):
                nc.vector.tensor_tensor(
                    sparts[:, 0:step, :], sparts[:, 0:step, :],
                    sparts[:, step:2 * step, :], OP.add)
            rt = pp.tile([128, NMT], f32, tag="rt")
            nc.scalar.activation(rt, sparts[:, 0, :], FT.Ln,
                                 bias=epsc[:, 0:1], scale=1.0 / HID)
            nc.scalar.activation(rt, rt, FT.Exp, scale=-0.5)

            # ---- out-projection --------------------------------------------
            zcs = []
            for kb in range(KB):
                zc = zl.tile([128, M], bf16, tag="zc")
                nc.sync.dma_start(out=zc,
                                  in_=agout_d[kb * CW:(kb + 1) * CW, 0:M])
                zcs.append(zc)
            for mb in range(NMB):
                yps = ps_proj.tile([128, MBS], f32, tag="proj")
                for kb in range(KB):
                    nc.tensor.matmul(yps, lhsT=woT[:, kb, :],
                                     rhs=zcs[kb][:, mb * MBS:(mb + 1) * MBS],
                                     start=(kb == 0), stop=(kb == KB - 1))
                ybf = fin.tile([128, MBS], bf16, tag="ybf")
                nc.vector.tensor_copy(ybf, yps)
                ytr = fin.tile([128, MBS // 128, 128], bf16, tag="ytr")
                nc.sync.dma_start(out=ytr, in_=ybf, transpose=True)
                for q in range(MBS // 128):
                    mt = mb * (MBS // 128) + q
                    yf = fin.tile([128, 128], f32, tag="yf")
                    nc.vector.tensor_scalar(yf, ytr[:, q, :],
                                            rt[:, mt:mt + 1], None, OP.mult)
                    nc.sync.dma_start(
                        out=y_d[mt * 128:(mt + 1) * 128, :], in_=yf)

    nc.compile()
    return nc


def _get_nc():
    if "nc" not in _CACHE:
        _CACHE["nc"] = _build_nc()
    return _CACHE["nc"]


def make_in_maps(x, Wq, Wk, Wv, Wo, Wg, Wog, Wd, bd, norm_w):
    xf = np.ascontiguousarray(np.asarray(x, np.float32).reshape(M, DM))
    Wq, Wk, Wv, Wo, Wg, Wog, Wd = (np.asarray(w, np.float32)
                                   for w in (Wq, Wk, Wv, Wo, Wg, Wog, Wd))
    bd = np.asarray(bd, np.float32)
    norm_w = np.asarray(norm_w, np.float32)
    maps = []
    for c in range(NCORE):
        cs = slice(c * CW, (c + 1) * CW)
        hs = slice(c * HL, (c + 1) * HL)
        maps.append({
            "x": xf,
            "wq": np.ascontiguousarray(Wq[cs]),
            "wk": np.ascontiguousarray(Wk[cs]),
            "wv": np.ascontiguousarray(Wv[cs]),
            "wg": np.ascontiguousarray(Wg[cs]),
            "wog": np.ascontiguousarray(Wog[cs]),
            "wo": np.ascontiguousarray(Wo[cs]),
            "wdt": np.ascontiguousarray(Wd[hs].T),
            "bd": np.ascontiguousarray(bd[hs].reshape(HL, 1)),
            "nw": np.ascontiguousarray(norm_w[cs].reshape(CW, 1)),
        })
    return maps


def kernel(x, Wq, Wk, Wv, Wo, Wg, Wog, Wd, bd, norm_w):
    from concourse import bass_utils
    nc = _get_nc()
    maps = make_in_maps(x, Wq, Wk, Wv, Wo, Wg, Wog, Wd, bd, norm_w)
    res = bass_utils.run_bass_kernel_spmd(nc, maps, core_ids=list(range(NCORE)))
    y = np.concatenate([np.asarray(res.results[c]["y"]) for c in range(NCORE)],
                       axis=1)
    return y.reshape(B, S, HID).astype(np.float32)
